# revision 19
# baseline (speedup 1.0000x reference)
"""Trainium2 Bass kernel for MinibatchDiscrimination2d (optimized v1).

Full computation:
  x (32,128,64,64) --conv s4--> x_r (32,3,16,16)
  M = x_r @ T  -> (32, 8192, 16)
  dist[b1,b2,d] = sum_f |M[b1,d,f]-M[b2,d,f]|
  out[b,d] = sum_b2 exp(-dist) - 1 -> (32,32,16,16)
  out_a = deconv s4 (32,32,64,64); return concat([x, out_a], ch)

Sharding over 8 cores (same as baseline): split t*t=256 output spatial
positions of D_OUT into 8 row-bands (2 of 16 t-rows per core). Conv is
data-parallel over B (4 samples/core) + AllGather of x_r (12KB).

v1 optimizations vs baseline (251us):
 - T shard stored fp8e3 (x64, clipped to +-15.5): halves HBM traffic and
   M-matmul is unchanged-cost (fp8 streams at bf16 rate).  x_r lhsT fp8e3
   (x4).  exp scale absorbs the 1/256.
 - M matmul col-tiled 4x: out (128 = 4 strips x 32 b, 512), each strip j
   covers T col-block j.  4x fewer PE cycles + 4x cheaper PSUM->SBUF copy.
 - D (pairwise) matmul row-tiled 2x per half-pass: lhsT = replicated sgn
   strips; Mb strips feed row-groups; 2 banks per half-pass, double-buffered.
 - Reduce split across engines: some (pc,half) units on DVE tensor_reduce
   (PSUM direct), the rest ACT Abs (PSUM->SBUF bf16) + DVE bf16 TT-add tree
   (2x mode).
 - accg/exp pipelined one g behind the matmul stream so the in-order PE
   queue never waits on ACT.
 - dummy AllGather issued at kernel start so CC bootstrap + core launch
   skew overlap conv/DMA instead of serializing.
 - y output bf16 (host converts to f32).

Per-core d index:  s = (r*16 + j)*32 + ch   (r in 0..1, j in 0..15, ch in 0..31)
dgroup g = s // 128; partition p = s % 128 = (rj%4)*32 + ch.
T shard col layout: g*2048 + p*16 + f.
"""

import numpy as np
import ml_dtypes

N_CORES = 8
B, IN_FLT, N = 32, 128, 64
K = 4
T_SP = 16
OC = 32
F = 16
D_IN = 768
BC = B // N_CORES          # 4 samples per core (conv data-parallel)
DSH = 1024                 # d per core
NG = DSH // 128            # 8 dgroups
KCH = D_IN // 128          # 6 contraction chunks

T_SCALE = 64.0
XR_SCALE = 4.0
FP8_MAX = 15.5             # e3m4 max normal
EXP_SCALE = 1.0 / (T_SCALE * XR_SCALE)

# reduce-route assignment per pair-chunk pc: listed pcs go ACT Abs +
# merged DVE bf16 tree; the others direct DVE tensor_reduce from PSUM.
ACT_PCS = (1, 3)

_CACHE = {}


def _build_nc():
    import concourse.bacc as bacc
    import concourse.mybir as mybir
    import concourse.tile as tile

    f32 = mybir.dt.float32
    bf16 = mybir.dt.bfloat16
    f8e3 = mybir.dt.float8e3
    AFT = mybir.ActivationFunctionType
    ALU = mybir.AluOpType
    AXL = mybir.AxisListType

    nc = bacc.Bacc("TRN2", target_bir_lowering=False, debug=False,
                   num_devices=N_CORES)

    xc = nc.dram_tensor("xc", [BC, IN_FLT, N, N], bf16, kind="ExternalInput")
    tsh = nc.dram_tensor("tsh", [D_IN, DSH * F], f8e3, kind="ExternalInput")
    wc = nc.dram_tensor("wc", [IN_FLT, 48], bf16, kind="ExternalInput")
    wd = nc.dram_tensor("wd", [OC, 512], bf16, kind="ExternalInput")
    eye = nc.dram_tensor("eye", [B, B], f32, kind="ExternalInput")
    sgnp = nc.dram_tensor("sgnp", [128, 512], bf16, kind="ExternalInput")
    inc = nc.dram_tensor("inc", [128, 128], bf16, kind="ExternalInput")
    y = nc.dram_tensor("y", [B, OC, 8, N], bf16, kind="ExternalOutput")

    from contextlib import ExitStack
    with tile.TileContext(nc) as tc, ExitStack() as stk:
            p = lambda *a, **kw: stk.enter_context(tc.tile_pool(*a, **kw))
            constp = p(name="const", bufs=1)
            dram = p(name="dram", bufs=1, space="DRAM")
            xbp = p(name="xb", bufs=1)
            Tp = p(name="Tp", bufs=3)
            Mp = p(name="Mp", bufs=2)
            absp = p(name="absp", bufs=2)
            t1p = p(name="t1p", bufs=2)
            t2p = p(name="t2p", bufs=2)
            t3p = p(name="t3p", bufs=2)
            distp = p(name="distp", bufs=2)
            egpp = p(name="egpp", bufs=2)
            ystp = p(name="ystp", bufs=2)
            pp = p(name="persist", bufs=1)
            ps_m = p(name="ps_m", bufs=2, space="PSUM")
            ps_d = p(name="ps_d", bufs=1, space="PSUM")
            ps_e = p(name="ps_e", bufs=1, space="PSUM")
            ps_x = p(name="ps_x", bufs=1, space="PSUM")

            wc_sb = constp.tile([IN_FLT, 48], bf16)
            nc.scalar.dma_start(wc_sb[:], wc[:])
            wd_sb = constp.tile([OC, 512], bf16)
            nc.scalar.dma_start(wd_sb[:], wd[:])
            eye_sb = constp.tile([B, B], f32)
            nc.scalar.dma_start(eye_sb[:], eye[:])
            sgn_sb = constp.tile([128, 512], bf16)
            nc.scalar.dma_start(sgn_sb[:], sgnp[:])
            inc_sb = constp.tile([128, 128], bf16)
            nc.scalar.dma_start(inc_sb[:], inc[:])

            # ---- Stage A: conv, col-tiled over the 4 local samples.
            # x loaded one sample at a time so conv starts early.
            xball = xbp.tile([IN_FLT, BC * N * N], bf16, tag="xb")
            xb_s = xball[:].rearrange("c (b hw) -> c b hw", b=BC)
            for smp in range(BC):
                nc.sync.dma_start(
                    xb_s[:, smp], xc[smp].rearrange("c h w -> c (h w)"))
            xb_rs = xball[:].rearrange(
                "c (b i r j s) -> c b r s i j", b=BC, i=16, r=4, j=16, s=4)
            psc_t = ps_x.tile([128, 512], f32, tag="x")
            psc = psc_t[:, :256]
            for smp in range(BC):
                for idx in range(16):
                    r, s = idx // 4, idx % 4
                    nc.tensor.matmul(
                        psc[32 * smp:32 * smp + 3, :].rearrange(
                            "p (i j) -> p i j", i=16),
                        wc_sb[:, idx * 3:idx * 3 + 3],
                        xb_rs[:, smp, r, s],
                        start=(idx == 0), stop=(idx == 15),
                        tile_position=(0, 32 * smp), skip_group_check=True)
            xrl = pp.tile([128, 256], f32)
            nc.vector.memset(xrl[:], 0.0)
            for smp in range(BC):
                nc.vector.tensor_copy(xrl[32 * smp:32 * smp + 3, :],
                                      psc[32 * smp:32 * smp + 3, :])

            ag_in = dram.tile([BC, D_IN], f32)
            ag_out = dram.tile([B, D_IN], f32)
            for smp in range(BC):
                nc.gpsimd.dma_start(
                    ag_in[smp].rearrange("(c ij) -> c ij", c=3),
                    xrl[32 * smp:32 * smp + 3, :])
            nc.gpsimd.collective_compute(
                "AllGather", ALU.bypass,
                replica_groups=[list(range(N_CORES))],
                ins=[ag_in.opt()], outs=[ag_out.opt()])

            # ---- Stage B: x_r^T chunks, scaled+clipped to fp8e3
            xr_all = pp.tile([B, D_IN], f32)
            nc.gpsimd.dma_start(xr_all[:], ag_out[:])
            xrT = pp.tile([128, KCH * B], f8e3)
            xrt_tmp = pp.tile([128, KCH * B], f32)
            for k in range(KCH):
                pst_t = ps_x.tile([128, 512], f32, tag="x")
                pst = pst_t[:, :B]
                nc.tensor.transpose(pst[:], xr_all[:, k * 128:(k + 1) * 128],
                                    eye_sb[:])
                tmp = xrt_tmp[:, k * B:(k + 1) * B]
                nc.vector.tensor_scalar(tmp, pst[:], XR_SCALE, FP8_MAX,
                                        ALU.mult, ALU.min)
                nc.vector.tensor_scalar(xrT[:, k * B:(k + 1) * B], tmp,
                                        -FP8_MAX, None, ALU.max)

            acc = pp.tile([128, NG * B], f32)        # col = g*32 + b
            acc2 = pp.tile([OC, 32 * B], bf16)       # (32 ch, col = rj*32 + b)
            wd_v = wd_sb[:].rearrange("c (m v) -> c v m", v=4)

            def _deconv_r(r):
                acc2_3 = acc2[:].rearrange("c (g x b) -> c g x b", g=NG, x=4)
                for q in range(4):
                    nc.gpsimd.dma_start(
                        acc2_3[:, 4 * r:4 * r + 4, q, :],
                        acc[q * 32:(q + 1) * 32, 4 * r * B:(4 * r + 4) * B]
                        .rearrange("c (g b) -> c g b", g=4))
                yst = ystp.tile([128, B * N], bf16)   # col = b*64 + 4j + v
                yst_r = yst[:].rearrange("p (b j v) -> p j b v", j=16, v=4)
                for v in range(4):
                    psd_t = ps_x.tile([128, 512], f32, tag="x")
                    nc.tensor.matmul(
                        psd_t[:], wd_v[:, v], acc2[:, r * 512:(r + 1) * 512],
                        start=True, stop=True)
                    nc.scalar.copy(
                        yst_r[:, :, :, v],
                        psd_t[:].rearrange("p (j b q) -> p j b q", j=16, q=1))
                for u in range(4):
                    nc.gpsimd.dma_start(
                        y[:, :, 4 * r + u, :].rearrange("b o c -> o b c"),
                        yst[u * 32:(u + 1) * 32, :]
                        .rearrange("o (b c) -> o b c", c=N))

            # ---- main loop: per dgroup g
            # Software-pipelined emission: M(g+1) matmuls are interleaved
            # with D(g) units so the in-order PE queue always has ready
            # work while DVE/ACT drain psD.  exp/accg run one g behind.
            # psD is one persistent 4-bank tile; subtile deps recycle the
            # two (128,1024) halves like a double buffer.
            psd_t = ps_d.tile([128, 2 * 1024], f32, tag="d")
            pending = []  # list of (g, dist2 tile)

            def _flush_pending():
                gprev, dist2 = pending.pop(0)
                Egp = egpp.tile([128, 512], bf16)
                nc.scalar.activation(Egp[:], dist2[:], AFT.Exp,
                                     scale=-EXP_SCALE)
                accg_t = ps_e.tile([128, 512], f32, tag="acc")
                accg = accg_t[:, :B]
                for pc in range(4):
                    nc.tensor.matmul(
                        accg, Egp[:, pc * 128:(pc + 1) * 128],
                        inc_sb[:, pc * B:(pc + 1) * B],
                        start=(pc == 0), stop=(pc == 3))
                nc.scalar.copy(acc[:, gprev * B:(gprev + 1) * B], accg)
                if gprev in (NG // 2 - 1, NG - 1):
                    _deconv_r(gprev // (NG // 2))

            def _emit_T(g):
                Tg = Tp.tile([128, KCH * 2048], f8e3, tag="T")
                nc.sync.dma_start(
                    Tg[:].rearrange("p (k c) -> p k c", k=KCH),
                    tsh[:, g * 2048:(g + 1) * 2048]
                    .rearrange("(k p) c -> p k c", k=KCH))
                return Tg

            def _emit_M_chunks(psm, Tg, ks):
                for k in ks:
                    for j in range(4):
                        nc.tensor.matmul(
                            psm[32 * j:32 * (j + 1), :],
                            xrT[:, k * B:(k + 1) * B],
                            Tg[:, k * 2048 + j * 512:k * 2048 + (j + 1) * 512],
                            start=(k == 0), stop=(k == KCH - 1),
                            tile_position=(0, 32 * j), skip_group_check=True)

            # chunk-pairs of next-g M work interleaved after each pc of D
            M_INTERLEAVE = {0: (0, 1), 1: (2, 3), 2: (4, 5), 3: ()}

            Tgs = {0: _emit_T(0), 1: _emit_T(1)}
            psm_cur = ps_m.tile([128, 512], f32, tag="m")
            _emit_M_chunks(psm_cur, Tgs[0], range(KCH))
            Mb_cur = Mp.tile([128, 512], bf16, tag="M")
            nc.scalar.copy(Mb_cur[:], psm_cur[:])

            for g in range(NG):
                if g + 2 < NG:
                    Tgs[g + 2] = _emit_T(g + 2)
                Mb = Mb_cur
                if g + 1 < NG:
                    psm_nxt = ps_m.tile([128, 512], f32, tag="m")
                dist2 = distp.tile([128, 512], bf16, tag="dist")
                for pc in range(4):
                    # next-g M matmuls first: they're always ready, so the
                    # in-order PE queue fills the time while DVE/ACT drain
                    # the previous pc's psD blocks.
                    if g + 1 < NG:
                        _emit_M_chunks(psm_nxt, Tgs[g + 1], M_INTERLEAVE[pc])
                    for h in range(2):
                        for i2 in range(2):
                            i = 2 * h + i2
                            nc.tensor.matmul(
                                psd_t[:, (2 * h + i2) * 512:
                                      (2 * h + i2 + 1) * 512],
                                sgn_sb[32 * i:32 * (i + 1),
                                       pc * 128:(pc + 1) * 128],
                                Mb[32 * i:32 * (i + 1), :],
                                start=True, stop=True,
                                tile_position=(32 * i, 0))
                    dsl = dist2[:, pc * 128:(pc + 1) * 128]
                    if pc in ACT_PCS:
                        absD = absp.tile([128, 2048], bf16, tag="a")
                        nc.scalar.activation(absD[:, :1024], psd_t[:, :1024],
                                             AFT.Abs)
                        nc.scalar.activation(absD[:, 1024:], psd_t[:, 1024:],
                                             AFT.Abs)
                        a3 = absD[:].rearrange("p (s f) -> p s f", f=16)
                        with nc.allow_low_precision(reason="dist bf16"):
                            t1 = t1p.tile([128, 1024], bf16, tag="t1")
                            t1r = t1[:].rearrange("p (s f) -> p s f", f=8)
                            nc.vector.tensor_tensor(
                                t1r, a3[:, :, 0:8], a3[:, :, 8:16], ALU.add)
                            t2 = t2p.tile([128, 512], bf16, tag="t2")
                            t2r = t2[:].rearrange("p (s f) -> p s f", f=4)
                            nc.vector.tensor_tensor(
                                t2r, t1r[:, :, 0:4], t1r[:, :, 4:8], ALU.add)
                            t3 = t3p.tile([128, 256], bf16, tag="t3")
                            t3r = t3[:].rearrange("p (s f) -> p s f", f=2)
                            nc.vector.tensor_tensor(
                                t3r, t2r[:, :, 0:2], t2r[:, :, 2:4], ALU.add)
                            nc.vector.tensor_tensor(
                                dsl.rearrange("p (s o) -> p s o", o=1),
                                t3r[:, :, 0:1], t3r[:, :, 1:2], ALU.add)
                    else:
                        with nc.allow_low_precision(reason="dist bf16"):
                            for h in range(2):
                                nc.vector.tensor_reduce(
                                    dsl[:, 64 * h:64 * h + 64],
                                    psd_t[:, h * 1024:(h + 1) * 1024]
                                    .rearrange("p (s f) -> p s f", f=F),
                                    axis=AXL.X, op=ALU.add,
                                    apply_absolute_value=True)
                pending.append((g, dist2))
                if g + 1 < NG:
                    Mb_cur = Mp.tile([128, 512], bf16, tag="M")
                    nc.scalar.copy(Mb_cur[:], psm_nxt[:])
                    psm_cur = psm_nxt
                if g >= 1:
                    _flush_pending()
            _flush_pending()

            # pin xball's live range to the end: the allocator otherwise
            # reuses its SBUF for Tg tiles without a DMA-write-after-PE-read
            # sync (race seen in MultiCoreSim).
            pin = constp.tile([1, 8], bf16)
            nc.vector.tensor_copy(pin[:], xball[0:1, 0:8])

    nc.finalize()
    return nc


def _host_prep(x, w_conv, T, w_deconv):
    """Build the 8 per-core input maps."""
    bf = ml_dtypes.bfloat16
    e3 = ml_dtypes.float8_e3m4
    # T: (768, 8192, 16) -> (768, 32ch, 16i, 16j, 16f)
    Tr = np.ascontiguousarray(T).reshape(D_IN, OC, T_SP, T_SP, F)
    wc_host = np.ascontiguousarray(
        np.transpose(w_conv, (1, 2, 3, 0)).reshape(IN_FLT, 48)).astype(bf)
    wd_host = np.ascontiguousarray(
        np.transpose(w_deconv, (1, 2, 0, 3)).reshape(OC, 512)).astype(bf)
    eye_host = np.eye(B, dtype=np.float32)

    # pairwise sign matrix (b1 < b2, 496 pairs padded to 512) and incidence
    pairs = [(a, b) for a in range(B) for b in range(a + 1, B)]
    sgn_host = np.zeros((B, 512), np.float32)
    inc_host = np.zeros((128, 128), np.float32)
    for p, (a, b) in enumerate(pairs):
        sgn_host[a, p] = 1.0
        sgn_host[b, p] = -1.0
        inc_host[p % 128, (p // 128) * B + a] = 1.0
        inc_host[p % 128, (p // 128) * B + b] = 1.0
    sgnp_host = np.tile(sgn_host, (4, 1)).astype(bf)      # (128, 512)
    inc_host = inc_host.astype(bf)

    in_maps = []
    for c in range(N_CORES):
        # shard: i rows 2c, 2c+1; column order s=(r*16+j)*32+ch, then f
        tslice = Tr[:, :, 2 * c:2 * c + 2, :, :]            # (768, ch, r, j, f)
        tshard = np.transpose(tslice, (0, 2, 3, 1, 4)).reshape(D_IN, DSH * F)
        tshard = np.clip(tshard * T_SCALE, -FP8_MAX, FP8_MAX).astype(e3)
        in_maps.append({
            "xc": np.ascontiguousarray(x[BC * c:BC * (c + 1)]).astype(bf),
            "tsh": np.ascontiguousarray(tshard),
            "wc": wc_host,
            "wd": wd_host,
            "eye": eye_host,
            "sgnp": sgnp_host,
            "inc": inc_host,
        })
    return in_maps


def _get_nc():
    if "nc" not in _CACHE:
        _CACHE["nc"] = _build_nc()
    return _CACHE["nc"]


def run(inputs, trace=False, trace_kwargs=None):
    """Run on hardware; returns (full_output, BassKernelResults)."""
    from concourse.bass_utils import run_bass_kernel_spmd
    nc = _get_nc()
    in_maps = _host_prep(inputs["x"], inputs["w_conv"], inputs["T"],
                         inputs["w_deconv"])
    res = run_bass_kernel_spmd(nc, in_maps, list(range(N_CORES)), trace=trace,
                               **(trace_kwargs or {}))
    x = np.asarray(inputs["x"], dtype=np.float32)
    full = np.empty((B, IN_FLT + OC, N, N), np.float32)
    full[:, :IN_FLT] = x
    for c in range(N_CORES):
        full[:, IN_FLT:, 8 * c:8 * (c + 1), :] = \
            res.results[c]["y"].astype(np.float32)
    return full, res


def kernel(**inputs) -> np.ndarray:
    out, _ = run(inputs, trace=False)
    return out
